# revision 52
# baseline (speedup 1.0000x reference)
"""Self-contained Trainium2 Bass kernel for BoSs (block-of-states) attention.

Strategy (8 NeuronCores):
  - data-parallel over batch (2) x tensor-parallel over heads (4):
    core c handles batch c//4, q-heads [4g:4g+4] and kv-head g where g=c%4.
  - host packs tokens by state id (stable sort) so the BoSs mask becomes
    block-banded causal in packed coordinates (max segment <=385 with
    BAND_BACK=3, so the WIN=1024 sliding window never binds).
  - Q/K/V and O projections run as 3-term hi/lo fp8 DoubleRow matmuls
    (y = xh@Wh + xl@Wh + xh@Wl at 0.75x the fp16 PE cost but ~fp16
    accuracy). Weights are pre-scaled by 32 so the fp8 lo-residuals stay
    out of e4m3's denormal range; the scale folds exactly into the exp
    scale, a 32-valued ones-vector for the softmax denominator, and a
    final /32 on the host.
  - scores/AV/denominator stay fp16; scores are computed transposed
    ([k, q]) so attention weights feed the AV matmul without transposes.
  - exp runs on Act with a uniform -3 bias (cancels in softmax, keeps
    1/l inside fp16 normal range); the BoSs mask is applied
    MULTIPLICATIVELY ({0,1} fp16) after exp on DVE's 4x mode.
  - V projection is computed directly transposed (vA[k,d] = x^T Wv^T).
  - weights are host-pre-arranged to their SBUF layouts (single
    full-bandwidth DMAs); masks load on the DVE queue, outputs store via
    the Pool/SWDGE queue, so the one shared HWDGE stays off the critical
    path.
  - emission pipelining: rope swaps and attention l/AV matmuls are
    emitted one step behind their producers so the PE always has
    independent work while Act/DVE catch up.
"""

import numpy as np
from contextlib import ExitStack

# problem constants (hardcoded per spec)
B, L, HID = 2, 2048, 2048
H, KVH, D = 16, 4, 128
THETA = 10000.0
NCORES = 8
TP = 4            # tensor-parallel group size (cores per batch)
QH = H // TP      # q heads per core = 4
QCH = 256         # q columns per attention chunk
NJQ = L // QCH    # 8
NKB = L // 128    # 16 k-blocks / q-blocks
NHC = HID // 128  # 16 hidden-dim chunks
LC = 512          # phase-1 L-chunk width
NLC = L // LC     # 4
BAND_BACK = 3     # k-block lookback; correct while max segment <= 385
SCALE = float(D) ** -0.5
WS = 32.0         # fp8 weight pre-scale (power of two, folded out exactly)
EXP_BIAS = -3.0   # uniform exp bias; cancels in softmax, centers 1/l in fp16


def _mbase(bands):
    return np.cumsum([0] + [hi - lo + 1 for lo, hi in bands]).tolist()


_CACHE = {}
LAST_EXEC_NS = None
LAST_RUN_WALL_S = None


def _build_nc(bands):
    import concourse.tile as tile
    from concourse import bacc, mybir

    f32 = mybir.dt.float32
    f16 = mybir.dt.float16
    f8 = mybir.dt.float8e4
    DR = mybir.MatmulPerfMode.DoubleRow
    EXP = mybir.ActivationFunctionType.Exp
    NBLK = sum(hi - lo + 1 for lo, hi in bands)
    MBASE = _mbase(bands)

    nc = bacc.Bacc(
        "TRN2", target_bir_lowering=False, debug=False, num_devices=NCORES
    )

    xTh = nc.dram_tensor("xTh", [HID, L], f8, kind="ExternalInput").ap()
    xTl = nc.dram_tensor("xTl", [HID, L], f8, kind="ExternalInput").ap()
    # weights pre-arranged to SBUF layout on host
    wqd = [nc.dram_tensor(f"wq{i}", [128, NHC, QH * D], f8,
                          kind="ExternalInput").ap() for i in range(2)]
    wkd = [nc.dram_tensor(f"wk{i}", [128, NHC, D], f8,
                          kind="ExternalInput").ap() for i in range(2)]
    wvd = [nc.dram_tensor(f"wv{i}", [128, NHC, D], f8,
                          kind="ExternalInput").ap() for i in range(2)]
    wod = [nc.dram_tensor(f"wo{i}", [128, QH, HID], f8,
                          kind="ExternalInput").ap() for i in range(2)]
    cosd = nc.dram_tensor("cosd", [D, L], f16, kind="ExternalInput").ap()
    sind = nc.dram_tensor("sind", [D, L], f16, kind="ExternalInput").ap()
    mskd = nc.dram_tensor("mskd", [128, NBLK, QCH], f16, kind="ExternalInput").ap()
    swpd = nc.dram_tensor("swpd", [128, 128], f16, kind="ExternalInput").ap()
    out = nc.dram_tensor("out", [L, HID], f16, kind="ExternalOutput").ap()

    with tile.TileContext(nc) as tc, ExitStack() as top:
        persist = top.enter_context(tc.tile_pool(name="persist", bufs=1))
        kT = persist.tile([128, L], f16, tag="kT", name="kT")
        qT = [
            persist.tile([128, L], f16, tag=f"qT{h}", name=f"qT{h}")
            for h in range(QH)
        ]
        # o in fp8 hi/lo pairs, head-major layout for DoubleRow O-proj pairs
        o8h = persist.tile([128, QH, L], f8, tag="o8h", name="o8h")
        o8l = persist.tile([128, QH, L], f8, tag="o8l", name="o8l")
        vA = persist.tile([128, NKB, 128], f16, tag="vA", name="vA")
        cosT = persist.tile([128, L], f16, tag="cosT", name="cosT")
        sinT = persist.tile([128, L], f16, tag="sinT", name="sinT")
        ones = persist.tile([128, 1], f16, tag="ones", name="ones")
        swp = persist.tile([128, 128], f16, tag="swp", name="swp")
        bias_t = persist.tile([128, 1], f32, tag="biast", name="bias_t")

        nc.vector.memset(ones[:], WS)
        nc.vector.memset(bias_t[:], EXP_BIAS)

        # weights / inputs (live whole kernel)
        wpool = top.enter_context(tc.tile_pool(name="wpool", bufs=1))
        wq_s = [wpool.tile([128, NHC, QH * D], f8, tag=f"wq{i}", name=f"wq_s{i}")
                for i in range(2)]
        wk_s = [wpool.tile([128, NHC, D], f8, tag=f"wk{i}", name=f"wk_s{i}")
                for i in range(2)]
        wv_s = [wpool.tile([128, NHC, D], f8, tag=f"wv{i}", name=f"wv_s{i}")
                for i in range(2)]
        wo_s = [wpool.tile([128, QH, HID], f8, tag=f"wo{i}", name=f"wo_s{i}")
                for i in range(2)]
        xpool = top.enter_context(tc.tile_pool(name="xpool", bufs=2))

        # ---- startup DMAs in need-order. SP queue: weights + x. Act queue:
        # rope tables + wo. HWDGE is one shared ~625ns/issue resource. ----
        xt0 = [xpool.tile([128, NHC, LC], f8, tag=f"x{i}", name=f"xt0_{i}")
               for i in range(2)]
        # x streams on the Pool/SWDGE queue (skips the shared HWDGE and
        # overlaps weight-DMA issue on SP); weights on SP; rope tables Act
        nc.sync.dma_start(wk_s[0][:], wkd[0][:])
        for q in range(4):
            nc.gpsimd.dma_start(
                xt0[0][:, 4 * q : 4 * q + 4, :],
                xTh[4 * q * 128 : (4 * q + 4) * 128, 0:LC].rearrange(
                    "(c p) n -> p c n", p=128),
            )
        nc.sync.dma_start(wk_s[1][:], wkd[1][:])
        nc.scalar.dma_start(cosT[:], cosd[:])
        for q in range(4):
            nc.gpsimd.dma_start(
                xt0[1][:, 4 * q : 4 * q + 4, :],
                xTl[4 * q * 128 : (4 * q + 4) * 128, 0:LC].rearrange(
                    "(c p) n -> p c n", p=128),
            )
        nc.sync.dma_start(wv_s[0][:], wvd[0][:])
        nc.sync.dma_start(wv_s[1][:], wvd[1][:])
        nc.scalar.dma_start(sinT[:], sind[:])
        nc.scalar.dma_start(swp[:], swpd[:])
        # Q weights per head-block, hi one head ahead of lo
        for hb in range(QH):
            for i in range(2):
                nc.sync.dma_start(
                    wq_s[i][:, :, hb * 128 : (hb + 1) * 128],
                    wqd[i][:, :, hb * 128 : (hb + 1) * 128],
                )
        # wo is needed only by the first O-proj (~40% in); its DMAs are
        # emitted inside the main loop so the 2MB doesn't block startup

        tpool = top.enter_context(tc.tile_pool(name="tpool", bufs=3))
        mpool = top.enter_context(tc.tile_pool(name="mpool", bufs=2))
        ppool = top.enter_context(tc.tile_pool(name="ppool", bufs=4))
        spool = top.enter_context(tc.tile_pool(name="spool", bufs=2))
        opool = top.enter_context(tc.tile_pool(name="opool", bufs=2))
        # PSUM: one shared rotating pool of 5 banks (projection chains, rope
        # swaps, V slots, O-proj, S sub-chunks) + AV out (2x1) + denoms (1).
        psA = top.enter_context(tc.tile_pool(name="psA", bufs=4, space="PSUM"))
        psO = top.enter_context(tc.tile_pool(name="psO", bufs=2, space="PSUM"))
        psL = top.enter_context(tc.tile_pool(name="psL", bufs=1, space="PSUM"))

        NCH = NHC // 2  # 8 DoubleRow K-steps (256-contraction each)
        # (x_tile, w_tile) 3-term order: shared x-lo second, per-target W-lo
        # last, so each chain's inputs arrive in emission order at startup
        TERMS = ((0, 0), (1, 0), (0, 1))

        def chain_part(ps, w_tiles, lhs_col0, xt, terms, first, last):
            # partial 3-term DoubleRow accumulation chains (both n0 halves)
            for n0 in (0, 256):
                for (xi, wi) in terms:
                    for c in range(NCH):
                        nc.tensor.matmul(
                            ps[:, n0 : n0 + 256],
                            w_tiles[wi][:, 2 * c : 2 * c + 2,
                                        lhs_col0 : lhs_col0 + 128],
                            xt[xi][:, 2 * c : 2 * c + 2, n0 : n0 + 256],
                            start=(first and (xi, wi) == terms[0] and c == 0),
                            stop=(last and (xi, wi) == terms[-1]
                                  and c == NCH - 1),
                            perf_mode=DR,
                        )

        def vchain_part(vps, xt, lc, terms, first, last):
            for kbl in range(4):
                for (xi, wi) in terms:
                    for c in range(NCH):
                        nc.tensor.matmul(
                            vps[:, kbl * 128 : kbl * 128 + 128],
                            xt[xi][:, 2 * c : 2 * c + 2,
                                   kbl * 128 : kbl * 128 + 128],
                            wv_s[wi][:, 2 * c : 2 * c + 2, :],
                            start=(first and (xi, wi) == terms[0] and c == 0),
                            stop=(last and (xi, wi) == terms[-1]
                                  and c == NCH - 1),
                            perf_mode=DR,
                        )

        def rope_finish(plain, sw, dst, cols, i):
            # dst[:, cols] = plain*cos + rotate_half(plain)*sin  (sw already
            # holds the swapped halves)
            t1 = tpool.tile([128, LC], f16, tag="t1", name=f"t1_{i}")
            nc.vector.tensor_mul(t1[:], plain[:], cosT[:, cols])
            t2 = tpool.tile([128, LC], f16, tag="t2", name=f"t2_{i}")
            nc.vector.tensor_mul(t2[:], sw[:], sinT[:, cols])
            nc.vector.tensor_add(dst[:, cols], t1[:], t2[:])

        def emit_proj(lc):
            cols = slice(lc * LC, (lc + 1) * LC)
            if lc == 0:
                xt = xt0
            else:
                xt = [xpool.tile([128, NHC, LC], f8, tag=f"x{i}",
                                 name=f"xt{lc}_{i}") for i in range(2)]
                nc.sync.dma_start(
                    xt[0][:], xTh[:, cols].rearrange("(c p) n -> p c n", p=128)
                )
                nc.sync.dma_start(
                    xt[1][:], xTl[:, cols].rearrange("(c p) n -> p c n", p=128)
                )
            # chains emitted with rope swaps one step behind, so the PE never
            # waits on the Act plain-copy of the chain it just finished
            pend = None  # (plain_tile, ps, dst)
            def flush_pend():
                nonlocal pend
                if pend is None:
                    return
                plain, ps, dst = pend
                sw = psA.tile([128, LC], f32, tag="A", name=f"sw{lc}_{id(dst)}")
                nc.tensor.matmul(sw[:], swp[:], plain[:], start=True, stop=True)
                rope_finish(plain, sw, dst, cols, f"{lc}_{id(dst)}")
                pend = None

            def close_target(ps, dst):
                nonlocal pend
                plain = tpool.tile([128, LC], f16, tag="plain",
                                   name=f"pl{lc}_{id(dst)}")
                nc.scalar.copy(plain[:], ps[:])
                flush_pend()
                pend = (plain, ps, dst)

            def copy_vA(vps):
                for kbl in range(4):
                    nc.scalar.copy(
                        vA[:, lc * 4 + kbl, :],
                        vps[:, kbl * 128 : kbl * 128 + 128],
                    )

            ps = psA.tile([128, LC], f32, tag="A", name=f"ppK{lc}")
            chain_part(ps, wk_s, 0, xt, TERMS, True, True)
            close_target(ps, kT)
            vps = psA.tile([128, LC], f32, tag="A", name=f"psv{lc}")
            vchain_part(vps, xt, lc, TERMS, True, True)
            copy_vA(vps)
            for hb in range(QH):
                ps = psA.tile([128, LC], f32, tag="A", name=f"pp{lc}_{hb}")
                chain_part(ps, wq_s, hb * 128, xt, TERMS, True, True)
                close_target(ps, qT[hb])
            flush_pend()

        def attn_head(jq, h, msk, l_ps, o_ps):
            # S + exp + mask for head h of chunk jq; returns P
            lo, hi = bands[jq]
            nkb = hi - lo + 1
            qs = slice(jq * QCH, (jq + 1) * QCH)
            P = ppool.tile([128, nkb, QCH], f16, tag="P", name=f"p{jq}_{h}")
            SB = 2
            for p0 in range(0, nkb, SB):
                pn = min(SB, nkb - p0)
                s_ps = psA.tile(
                    [128, SB, QCH], f32, tag="A", name=f"s{jq}_{h}_{p0}"
                )
                for i in range(p0, p0 + pn):
                    kb = lo + i
                    nc.tensor.matmul(
                        s_ps[:, i - p0, :],
                        kT[:, kb * 128 : (kb + 1) * 128],
                        qT[h][:, qs],
                        start=True,
                        stop=True,
                    )
                nc.scalar.activation(
                    P[:, p0 : p0 + pn, :],
                    s_ps[:, :pn, :],
                    EXP,
                    scale=SCALE / (WS * WS),
                    bias=bias_t[:],
                )
                nc.vector.tensor_mul(
                    P[:, p0 : p0 + pn, :],
                    P[:, p0 : p0 + pn, :],
                    msk[:, p0 : p0 + pn, :],
                )
            return P

        def attn_lav(jq, h, P, l_ps, o_ps):
            lo, hi = bands[jq]
            nkb = hi - lo + 1
            # denominator: 32-valued ones folds the 32x of vA out of 1/l;
            # one column slot per head of the pair, all at partition 0
            l_ap = l_ps[0:1, h % 2, :]
            for i in range(nkb):
                nc.tensor.matmul(
                    l_ap, ones[:], P[:, i, :],
                    start=(i == 0), stop=(i == nkb - 1),
                )
            for i in range(nkb):
                nc.tensor.matmul(
                    o_ps[:, h % 2, :],
                    vA[:, lo + i, :],
                    P[:, i, :],
                    start=(i == 0),
                    stop=(i == nkb - 1),
                )

        def attn_norm2(jq, pair, l_ps, o_ps):
            # normalization for one head-pair (starts as soon as that pair's
            # denominators are done, overlapping the other pair's l/AV)
            qs = slice(jq * QCH, (jq + 1) * QCH)
            rc = spool.tile([1, 2, QCH], f16, tag="rc", name=f"rc{jq}_{pair}")
            with nc.allow_low_precision(
                reason="fp16 1/l scales fp16 outputs; 5e-4 rel ok"
            ):
                nc.vector.reciprocal(rc[:], l_ps[0:1, :, :])
            for hh in range(2):
                h = 2 * pair + hh
                r_bc = spool.tile(
                    [128, QCH], f16, tag="lbc", bufs=4, name=f"lb{jq}_{h}"
                )
                nc.gpsimd.partition_broadcast(r_bc[:], rc[0:1, hh, :])
                t16 = tpool.tile([128, QCH], f16, tag="t16",
                                 name=f"t16_{jq}_{h}")
                nc.vector.tensor_mul(t16[:], o_ps[:, hh, :], r_bc[:])
                nc.scalar.copy(o8h[:, h, qs], t16[:])
                nc.gpsimd.tensor_sub(o8l[:, h, qs], t16[:], o8h[:, h, qs])

        def emit_attn(jq):
            lo, hi = bands[jq]
            nkb = hi - lo + 1
            msk = mpool.tile([128, nkb, QCH], f16, tag="m", name=f"msk{jq}")
            nc.scalar.dma_start(
                msk[:], mskd[:, MBASE[jq] : MBASE[jq] + nkb, :]
            )
            l_ps = [psL.tile([128, 2, QCH], f32, tag="l", name=f"l{jq}_{i}")
                    for i in range(2)]
            o_ps01 = psO.tile([128, 2, QCH], f32, tag="o", name=f"oA{jq}")
            o_ps23 = psO.tile([128, 2, QCH], f32, tag="o", name=f"oB{jq}")
            # l/AV emitted one head behind S/exp/mask so the PE overlaps the
            # Act/DVE latency with the next head's score matmuls; each
            # head-pair's normalization starts as soon as its l's are done
            Ps = []
            for h in range(QH):
                Ps.append(attn_head(jq, h, msk, l_ps, None))
                if h > 0:
                    attn_lav(jq, h - 1, Ps[h - 1],
                             l_ps[(h - 1) // 2],
                             o_ps01 if h - 1 < 2 else o_ps23)
                if h == 2:
                    attn_norm2(jq, 0, l_ps[0], o_ps01)
            attn_lav(jq, 3, Ps[3], l_ps[1], o_ps23)
            attn_norm2(jq, 1, l_ps[1], o_ps23)

        def emit_oproj(qb):
            ob = opool.tile([128, HID], f16, tag="ob", name=f"ob{qb}")
            qsl = slice(qb * 128, (qb + 1) * 128)
            for hc in range(HID // 512):
                f_ps = psA.tile([128, 512], f32, tag="A", name=f"f{qb}_{hc}")
                for n0 in range(0, 512, 256):
                    col = slice(hc * 512 + n0, hc * 512 + n0 + 256)
                    terms = ((o8h, 0), (o8l, 0), (o8h, 1))
                    for ti, (ot, wi) in enumerate(terms):
                        for hh in range(2):
                            nc.tensor.matmul(
                                f_ps[:, n0 : n0 + 256],
                                ot[:, 2 * hh : 2 * hh + 2, qsl],
                                wo_s[wi][:, 2 * hh : 2 * hh + 2, col],
                                start=(ti == 0 and hh == 0),
                                stop=(ti == 2 and hh == 1),
                                perf_mode=DR,
                            )
                # PSUM->SBUF copies split across Act/DVE to balance load
                if hc % 2 == 0:
                    nc.scalar.copy(ob[:, hc * 512 : (hc + 1) * 512], f_ps[:])
                else:
                    nc.vector.tensor_copy(ob[:, hc * 512 : (hc + 1) * 512], f_ps[:])
                # out stores in halves on SP (HWDGE is mostly idle);
                # quarters for the last two blocks to shorten the drain
                if qb >= 14:
                    nc.sync.dma_start(
                        out[qb * 128 : (qb + 1) * 128,
                            hc * 512 : (hc + 1) * 512],
                        ob[:, hc * 512 : (hc + 1) * 512],
                    )
                elif hc % 2 == 1:
                    nc.sync.dma_start(
                        out[qb * 128 : (qb + 1) * 128,
                            (hc - 1) * 512 : (hc + 1) * 512],
                        ob[:, (hc - 1) * 512 : (hc + 1) * 512],
                    )

        # O-proj for the second attention chunk of each lc is deferred past
        # the next projection chunk, hiding the normalization-chain latency
        pending = []
        for lc in range(NLC):
            emit_proj(lc)
            for qb in pending:
                emit_oproj(qb)
            pending = []
            emit_attn(2 * lc)
            if lc == 0:
                nc.scalar.dma_start(wo_s[0][:], wod[0][:])
                nc.scalar.dma_start(wo_s[1][:], wod[1][:])
            emit_attn(2 * lc + 1)
            emit_oproj(4 * lc)
            emit_oproj(4 * lc + 1)
            pending = [4 * lc + 2, 4 * lc + 3]
        for qb in pending:
            emit_oproj(qb)

    nc.compile()
    return nc


def _get_nc(bands):
    if bands not in _CACHE:
        _CACHE[bands] = _build_nc(bands)
    return _CACHE[bands]


def kernel(hidden_states, Wq, Wk, Wv, Wo, sid, position_ids):
    global LAST_EXEC_NS, LAST_RUN_WALL_S
    import time

    import ml_dtypes
    from concourse.bass_utils import run_bass_kernel_spmd

    f8 = ml_dtypes.float8_e4m3
    f16 = np.float16

    hidden = np.asarray(hidden_states, dtype=np.float32)
    Wq = np.asarray(Wq, dtype=np.float32)
    Wk = np.asarray(Wk, dtype=np.float32)
    Wv = np.asarray(Wv, dtype=np.float32)
    Wo = np.asarray(Wo, dtype=np.float32)
    sid = np.asarray(sid)
    position_ids = np.asarray(position_ids)


    def split8(a):
        h = a.astype(f8)
        l = (a - h.astype(np.float32)).astype(f8)
        return h, l

    def sb_layout(wT):
        # [HID, M] -> [128, NHC, M] SBUF layout, contiguous
        M = wT.shape[1]
        return np.ascontiguousarray(wT.reshape(NHC, 128, M).transpose(1, 0, 2))

    def wo_layout(woT):
        # [QH*D, HID] -> [128, QH, HID]
        return np.ascontiguousarray(woT.reshape(QH, 128, HID).transpose(1, 0, 2))

    swpm = np.zeros((128, 128), f16)
    swpm[(np.arange(128) + 64) % 128, np.arange(128)] = 1.0

    # data-dependent attention bands: per q-chunk, look back to the start
    # block of the chunk-first token's segment (union over both batches)
    bands = []
    sts = []
    for b in range(B):
        st = np.sort(sid[b].astype(np.int64))
        sts.append(st)
    for jq in range(NJQ):
        lo = NKB
        for b in range(B):
            st = sts[b]
            s0 = int(np.searchsorted(st, st[jq * QCH], side="left"))
            lo = min(lo, s0 // 128)
        bands.append((lo, 2 * jq + 1))
    bands = tuple(bands)
    MBASE = _mbase(bands)
    NBLK = sum(hi - lo + 1 for lo, hi in bands)

    nc = _get_nc(bands)

    in_maps = []
    perms = []
    for b in range(B):
        s = sid[b].astype(np.int64)
        perm = np.argsort(s, kind="stable")
        perms.append(perm)
        st = s[perm]

        pos = position_ids[b][perm].astype(np.float32)
        inv = (
            1.0
            / (THETA ** (np.arange(0, D, 2, dtype=np.float32) / np.float32(D)))
        ).astype(np.float32)
        fr = pos[:, None] * inv[None, :]
        emb = np.concatenate([fr, fr], axis=1)  # [L, D]
        cosT = np.ascontiguousarray(np.cos(emb).T.astype(f16))
        sinT = np.sin(emb).T.astype(np.float32).copy()
        sinT[: D // 2] *= -1.0  # fold rotate_half sign
        sinT = np.ascontiguousarray(sinT.astype(f16))

        xT = hidden[b].T[:, perm]
        xh, xl = split8(xT)
        xh, xl = np.ascontiguousarray(xh), np.ascontiguousarray(xl)

        # mask pre-arranged to [128, NBLK, QCH] SBUF layout
        msk = np.zeros((128, NBLK, QCH), f16)
        ki = np.arange(128)
        qi = np.arange(QCH)
        for jq in range(NJQ):
            lo, hi = bands[jq]
            for i in range(hi - lo + 1):
                kb = lo + i
                kidx = kb * 128 + ki
                qidx = jq * QCH + qi
                m = (st[kidx][:, None] == st[qidx][None, :]) & (
                    kidx[:, None] <= qidx[None, :]
                )
                msk[:, MBASE[jq] + i, :] = m.astype(f16)

        for g in range(TP):
            wqh, wql = split8(Wq[g * 512 : (g + 1) * 512].T * WS)
            wkh, wkl = split8(Wk[g * 128 : (g + 1) * 128].T * WS)
            wvh, wvl = split8(Wv[g * 128 : (g + 1) * 128].T * WS)
            woh, wol = split8(Wo[:, g * 512 : (g + 1) * 512].T * WS)
            in_maps.append(
                dict(
                    xTh=xh, xTl=xl,
                    wq0=sb_layout(wqh), wq1=sb_layout(wql),
                    wk0=sb_layout(wkh), wk1=sb_layout(wkl),
                    wv0=sb_layout(wvh), wv1=sb_layout(wvl),
                    wo0=wo_layout(woh), wo1=wo_layout(wol),
                    cosd=cosT,
                    sind=sinT,
                    mskd=msk,
                    swpd=swpm,
                )
            )

    t0 = time.time()
    res = run_bass_kernel_spmd(nc, in_maps, core_ids=list(range(NCORES)))
    LAST_RUN_WALL_S = time.time() - t0
    LAST_EXEC_NS = res.exec_time_ns

    full = np.empty((B, L, HID), np.float32)
    for b in range(B):
        acc = np.asarray(res.results[4 * b]["out"]).astype(np.float32)
        for g in range(1, TP):
            acc += np.asarray(res.results[4 * b + g]["out"]).astype(np.float32)
        acc /= np.float32(WS)
        unp = np.empty_like(acc)
        unp[perms[b]] = acc
        full[b] = unp
    return full
